# revision 1
# baseline (speedup 1.0000x reference)
"""Causal varlen self-attention (packed equal-length sequences) on 8 trn2 cores.

Sharding: 4 sequences x 2 head-groups. Core c handles sequence b = c//2 and
heads hh*8..hh*8+8 where hh = c%2. Each core computes QKV projection for its
sequence restricted to its heads, rotary+RMSNorm, causal attention for all
1024 rows of the sequence over its 8 heads, and a partial output projection
over its 512 features. The pair of cores for a sequence all-reduce their
partial y so every core ends with the full [1024, 1024] output of its
sequence; the host slices core 2b's output.

All matmuls run in float32r (TF32-like, ~1e-4 rel err, 3.4x faster than f32).
Softmax uses exp without max subtraction (RMS-normed q,k bound scores to
|s| <= 8) in a transposed scores layout [kpos, q], which avoids transposing
the probabilities for the PV matmul. Denominators come from a ones column
appended to V; per-head normalization happens on the small attention output.
"""
import numpy as np

N_EMBD = 1024
N_HEAD = 16
HD = 64
S = 1024
B = 4
N = B * S
NCORES = 8
HPC = 8           # heads per core
NHC = HPC // 2    # head-pair chunks per core
NB = S // 128     # row blocks per sequence
ND = N_EMBD // 128  # contraction chunks
JW = 3 * HPC * HD   # qkv feature width per core (1536)
NEG = -30000.0
RMS_EPS = 1.1920929e-07

_cached = {}


def _build():
    import concourse.bacc as bacc
    import concourse.mybir as mybir
    import concourse.tile as tile
    import concourse.bass as bass
    from concourse.masks import make_identity

    F32 = mybir.dt.float32
    F32R = mybir.dt.float32r

    nc = bacc.Bacc('TRN2', target_bir_lowering=False, debug=False,
                   num_devices=NCORES)
    xs = nc.dram_tensor('xs', [S, N_EMBD], F32, kind='ExternalInput').ap()
    wqkvT = nc.dram_tensor('wqkvT', [N_EMBD, JW], F32, kind='ExternalInput').ap()
    woT = nc.dram_tensor('woT', [HPC * HD, N_EMBD], F32, kind='ExternalInput').ap()
    cosg = nc.dram_tensor('cosg', [S, HD // 2], F32, kind='ExternalInput').ap()
    sing = nc.dram_tensor('sing', [S, HD // 2], F32, kind='ExternalInput').ap()
    ypart = nc.dram_tensor('ypart', [S, N_EMBD], F32, kind='ExternalOutput').ap()
    ystage = nc.dram_tensor('ystage', [S, N_EMBD], F32).ap()
    yred = nc.dram_tensor('yred', [S, N_EMBD], F32).ap()

    def bcast_mid(t, n, width):
        # view [128, width] tile as [128, n, width] broadcasting over middle dim
        return bass.AP(tensor=t.tensor, offset=t.offset,
                       ap=[t.ap[0], [0, n], t.ap[-1]])

    def bcast_last(t, width):
        # view [128, n] tile as [128, n, width] broadcasting over last dim
        return bass.AP(tensor=t.tensor, offset=t.offset,
                       ap=[t.ap[0], t.ap[1], [0, width]])

    with tile.TileContext(nc) as tc:
        import contextlib
        ctx = contextlib.ExitStack()
        with ctx:
            const = ctx.enter_context(tc.tile_pool(name='const', bufs=1))
            persist = ctx.enter_context(tc.tile_pool(name='persist', bufs=1))

            ident = const.tile([128, 128], F32)
            make_identity(nc, ident)
            # additive causal maskT[k, q] = 0 if k <= q else NEG
            maskT = const.tile([128, 128], F32)
            nc.gpsimd.memset(maskT, 0.0)
            nc.gpsimd.affine_select(
                out=maskT, in_=maskT, compare_op=mybir.AluOpType.is_ge,
                fill=NEG, base=0, pattern=[[1, 128]], channel_multiplier=-1)
            epst = const.tile([128, 1], F32)
            nc.vector.memset(epst, RMS_EPS)

            qT = [persist.tile([128, S], F32R, name=f'qT{i}') for i in range(NHC)]
            kT = [persist.tile([128, S], F32R, name=f'kT{i}') for i in range(NHC)]
            # v per head padded to 128 cols: cols 0:64 = v, 64:128 = ones, so the
            # PV matmul also produces 64 replicated denominator rows (free-dim
            # streaming cost is unchanged by M).
            vt = [persist.tile([128, HPC, 128], F32R, name=f'vt{i}') for i in range(NB)]
            attT = [persist.tile([128, S], F32R, name=f'attT{i}') for i in range(NHC)]

            # ---- phase 1+2: x transpose and QKV projection ----
            with tc.tile_pool(name='xtp', bufs=1) as xtp, \
                 tc.tile_pool(name='wqp', bufs=1) as wqp, \
                 tc.tile_pool(name='qkvwork', bufs=2) as qw, \
                 tc.tile_pool(name='scratch', bufs=2) as scratch, \
                 tc.tile_pool(name='pst', bufs=2, space='PSUM') as pst, \
                 tc.tile_pool(name='psq', bufs=2, space='PSUM') as psq:
                xT = [xtp.tile([128, S], F32R, name=f'xT{d}') for d in range(ND)]
                JH = JW // 2  # j-half width: [q | k0-3] then [k4-7 | v]

                for nb in range(NB):
                    xrow = qw.tile([128, N_EMBD], F32, tag='xrow')
                    nc.sync.dma_start(out=xrow, in_=xs[nb * 128:(nb + 1) * 128])
                    for d in range(ND):
                        pt = pst.tile([128, 128], F32, tag='pt')
                        nc.tensor.transpose(pt, xrow[:, d * 128:(d + 1) * 128], ident)
                        nc.vector.tensor_copy(xT[d][:, nb * 128:(nb + 1) * 128], pt)

                cost = [const.tile([128, HD // 2], F32, name=f'cos{i}') for i in range(NB)]
                sint = [const.tile([128, HD // 2], F32, name=f'sin{i}') for i in range(NB)]
                for nb in range(NB):
                    nc.sync.dma_start(out=cost[nb], in_=cosg[nb * 128:(nb + 1) * 128])
                    nc.sync.dma_start(out=sint[nb], in_=sing[nb * 128:(nb + 1) * 128])

                def rotary_rms(src, heads, is_q, cb, sb):
                    # src: [128, len(heads), 64] f32 view; returns normalized tile
                    nh = src.shape[1]
                    x1 = src[:, :, 0:32]
                    x2 = src[:, :, 32:64]
                    rot = scratch.tile([128, nh, HD], F32, tag='rot')
                    ra = scratch.tile([128, nh, 32], F32, tag='ra')
                    rb = scratch.tile([128, nh, 32], F32, tag='rb')
                    nc.vector.tensor_mul(ra, x1, cb)
                    nc.vector.tensor_mul(rb, x2, sb)
                    nc.vector.tensor_add(rot[:, :, 0:32], ra, rb)
                    nc.vector.tensor_mul(ra, x2, cb)
                    nc.vector.tensor_mul(rb, x1, sb)
                    nc.vector.tensor_tensor(out=rot[:, :, 32:64], in0=ra, in1=rb,
                                            op=mybir.AluOpType.subtract)
                    sq = scratch.tile([128, nh, HD], F32, tag='sq')
                    nc.vector.tensor_mul(sq, rot, rot)
                    ms = scratch.tile([128, nh], F32, tag='ms')
                    nc.vector.reduce_sum(out=ms, in_=sq, axis=mybir.AxisListType.X)
                    nc.scalar.activation(out=ms, in_=ms,
                                         func=mybir.ActivationFunctionType.Sqrt,
                                         bias=epst, scale=1.0 / HD)
                    nc.vector.reciprocal(out=ms, in_=ms)
                    if is_q:
                        nc.scalar.mul(out=ms, in_=ms, mul=HD ** -0.5)
                    nc.vector.tensor_mul(rot, rot, bcast_last(ms, HD))
                    return rot

                for jh in range(2):
                    wq = []
                    for d in range(ND):
                        wq32 = qw.tile([128, JH], F32, tag='wq32')
                        nc.sync.dma_start(
                            out=wq32, in_=wqkvT[d * 128:(d + 1) * 128, jh * JH:(jh + 1) * JH])
                        wqd = wqp.tile([128, JH], F32R, tag=f'wq{d}', name=f'wq{jh}_{d}')
                        nc.vector.tensor_copy(wqd, wq32)
                        wq.append(wqd)
                    for nb in range(NB):
                        pq = psq.tile([128, JH], F32, tag='pq')
                        for d in range(ND):
                            nc.tensor.matmul(
                                pq[:, 0:512],
                                xT[d][:, nb * 128:(nb + 1) * 128],
                                wq[d][:, 0:512],
                                start=(d == 0), stop=(d == ND - 1))
                            nc.tensor.matmul(
                                pq[:, 512:JH],
                                xT[d][:, nb * 128:(nb + 1) * 128],
                                wq[d][:, 512:JH],
                                start=(d == 0), stop=(d == ND - 1))
                        qkvs = qw.tile([128, JH // HD, HD], F32, tag='qkvs')
                        nc.vector.tensor_copy(qkvs, pq)

                        cb2 = bcast_mid(cost[nb], HPC, HD // 2)
                        sb2 = bcast_mid(sint[nb], HPC, HD // 2)
                        cb1 = bcast_mid(cost[nb], HPC // 2, HD // 2)
                        sb1 = bcast_mid(sint[nb], HPC // 2, HD // 2)
                        if jh == 0:
                            # q heads 0-7 then k heads 0-3
                            rotq = rotary_rms(qkvs[:, 0:HPC, :], HPC, True, cb2, sb2)
                            for hc in range(NHC):
                                pt2 = pst.tile([128, 128], F32, tag='pt')
                                nc.tensor.transpose(
                                    pt2, rotq[:, hc * 2:(hc + 1) * 2, :].rearrange("p a b -> p (a b)"),
                                    ident)
                                nc.vector.tensor_copy(qT[hc][:, nb * 128:(nb + 1) * 128], pt2)
                            rotk = rotary_rms(qkvs[:, HPC:HPC + 4, :], 4, False, cb1, sb1)
                            for hc in range(2):
                                pt2 = pst.tile([128, 128], F32, tag='pt')
                                nc.tensor.transpose(
                                    pt2, rotk[:, hc * 2:(hc + 1) * 2, :].rearrange("p a b -> p (a b)"),
                                    ident)
                                nc.vector.tensor_copy(kT[hc][:, nb * 128:(nb + 1) * 128], pt2)
                        else:
                            # k heads 4-7 then v heads 0-7
                            rotk = rotary_rms(qkvs[:, 0:4, :], 4, False, cb1, sb1)
                            for hc in range(2):
                                pt2 = pst.tile([128, 128], F32, tag='pt')
                                nc.tensor.transpose(
                                    pt2, rotk[:, hc * 2:(hc + 1) * 2, :].rearrange("p a b -> p (a b)"),
                                    ident)
                                nc.vector.tensor_copy(kT[2 + hc][:, nb * 128:(nb + 1) * 128], pt2)
                            nc.vector.tensor_copy(out=vt[nb][:, :, 0:HD], in_=qkvs[:, 4:4 + HPC, :])
                            nc.vector.memset(vt[nb][:, :, HD:128].bitcast(F32), 1.0)

            # ---- phase 3+4: attention interleaved with output projection ----
            with tc.tile_pool(name='estp', bufs=4) as estp, \
                 tc.tile_pool(name='attw', bufs=2) as attw, \
                 tc.tile_pool(name='wop', bufs=1) as wop, \
                 tc.tile_pool(name='ywork', bufs=3) as yw, \
                 tc.tile_pool(name='pssc', bufs=2, space='PSUM') as pssc, \
                 tc.tile_pool(name='pspv', bufs=1, space='PSUM') as pspv, \
                 tc.tile_pool(name='psy', bufs=2, space='PSUM') as psy:
                wo = [wop.tile([128, N_EMBD], F32R, name=f'wo{f}') for f in range(NHC)]
                for f in range(NHC):
                    wo32 = yw.tile([128, N_EMBD], F32, tag='wo32')
                    nc.sync.dma_start(out=wo32, in_=woT[f * 128:(f + 1) * 128])
                    nc.vector.tensor_copy(wo[f], wo32)
                for qg in range(2):
                    for hc in range(NHC):
                        nkc = 4 + qg * 4
                        pvs = [pspv.tile([128, 512], F32, name=f'pv{qg}_{hc}_{h2}', tag=f'pv{h2}')
                               for h2 in range(2)]
                        for kc in range(nkc):
                            vs = max(0, kc - qg * 4) * 128
                            diag = kc >= qg * 4
                            # both heads in one 2-bank psum tile
                            sct = pssc.tile([128, 2, 512], F32, tag='sc')
                            for h2 in range(2):
                                nc.tensor.matmul(
                                    sct[:, h2],
                                    kT[hc][h2 * HD:(h2 + 1) * HD, kc * 128:(kc + 1) * 128],
                                    qT[hc][h2 * HD:(h2 + 1) * HD, qg * 512:(qg + 1) * 512],
                                    start=True, stop=True,
                                    tile_position=(h2 * HD, 0))
                            if diag:
                                nc.vector.tensor_add(
                                    sct[:, :, vs:vs + 128], sct[:, :, vs:vs + 128],
                                    bcast_mid(maskT, 2, 128))
                            est = estp.tile([128, 2, 512], F32R, tag='est')
                            if vs > 0:
                                nc.vector.memset(est[:, :, 0:vs].bitcast(F32), 0.0)
                            nc.scalar.activation(out=est[:, :, vs:], in_=sct[:, :, vs:],
                                                 func=mybir.ActivationFunctionType.Exp)
                            for h2 in range(2):
                                nc.tensor.matmul(
                                    pvs[h2], vt[kc][:, hc * 2 + h2], est[:, h2],
                                    start=(kc == 0), stop=(kc == nkc - 1))
                        for h2 in range(2):
                            denr = attw.tile([HD, 512], F32, tag='denr')
                            nc.vector.reciprocal(denr, pvs[h2][HD:128, :])
                            nc.vector.tensor_mul(
                                attT[hc][h2 * HD:(h2 + 1) * HD, qg * 512:(qg + 1) * 512],
                                pvs[h2][0:HD, :], denr)

                    # project this half's rows while the other half's attention runs
                    for qt in range(qg * 4, qg * 4 + 4):
                        for og in range(2):
                            py = psy.tile([128, 512], F32, tag='py')
                            for f in range(NHC):
                                nc.tensor.matmul(
                                    py,
                                    attT[f][:, qt * 128:(qt + 1) * 128],
                                    wo[f][:, og * 512:(og + 1) * 512],
                                    start=(f == 0), stop=(f == NHC - 1))
                            ys = yw.tile([128, 512], F32, tag='ys')
                            nc.vector.tensor_copy(ys, py)
                            nc.sync.dma_start(
                                out=ystage[qt * 128:(qt + 1) * 128, og * 512:(og + 1) * 512],
                                in_=ys)
                    rs = slice(qg * 512, (qg + 1) * 512)
                    nc.gpsimd.collective_compute(
                        "AllReduce", mybir.AluOpType.add,
                        replica_groups=[[0, 1], [2, 3], [4, 5], [6, 7]],
                        ins=[ystage[rs, :]], outs=[yred[rs, :]])
                    nc.sync.dma_start(out=ypart[rs, :], in_=yred[rs, :])

    nc.compile()
    return nc


def _get_nc():
    if 'nc' not in _cached:
        _cached['nc'] = _build()
    return _cached['nc']


def kernel(x, Wqkv, Wo, cos_cache, sin_cache, cu_seqlens, position_ids,
           max_seqlen, **_ignored):
    from concourse.bass_utils import run_bass_kernel_spmd

    x = np.asarray(x)
    Wqkv = np.asarray(Wqkv)
    Wo = np.asarray(Wo)
    cos_cache = np.asarray(cos_cache)
    sin_cache = np.asarray(sin_cache)
    position_ids = np.asarray(position_ids)

    nc = _get_nc()
    in_maps = []
    for c in range(NCORES):
        b, hh = c // 2, c % 2
        rows = slice(b * S, (b + 1) * S)
        qsl = slice(hh * HPC * HD, (hh + 1) * HPC * HD)
        ksl = slice(N_EMBD + hh * HPC * HD, N_EMBD + (hh + 1) * HPC * HD)
        vsl = slice(2 * N_EMBD + hh * HPC * HD, 2 * N_EMBD + (hh + 1) * HPC * HD)
        wqkvT_c = np.concatenate(
            [Wqkv[qsl], Wqkv[ksl], Wqkv[vsl]], axis=0).T.copy()
        woT_c = Wo[:, qsl].T.copy()
        pos = position_ids[rows]
        in_maps.append({
            'xs': np.ascontiguousarray(x[rows]),
            'wqkvT': np.ascontiguousarray(wqkvT_c),
            'woT': np.ascontiguousarray(woT_c),
            'cosg': np.ascontiguousarray(cos_cache[pos]),
            'sing': np.ascontiguousarray(sin_cache[pos]),
        })

    r = run_bass_kernel_spmd(nc, in_maps, list(range(NCORES)))
    out = np.empty((N, N_EMBD), dtype=np.float32)
    for b in range(B):
        out[b * S:(b + 1) * S] = r.results[2 * b]['ypart']
    _cached['last_results'] = r
    return out



# revision 12
# speedup vs baseline: 1.5329x; 1.5329x over previous
"""Causal varlen self-attention (packed equal-length sequences) on 8 trn2 cores.

Sharding: 4 sequences x 2 head-groups. Core c handles sequence b = c//2 and
heads hh*8..hh*8+8 where hh = c%2. Each core computes the QKV projection for
its sequence restricted to its heads, rotary+RMSNorm, causal attention over
its 8 heads, and a partial output projection over its 512 features. The pair
of cores for a sequence ReduceScatter their partial y (each ends with half
the reduced rows); the host stitches the halves.

All matmuls run in bf16 (rel err ~4e-3, tolerance 2e-2). The host ships x
pre-transposed and weights pre-converted to bf16, so the device does no
f32 casts for weights and no x transpose. Attention computes only the
at-or-below-diagonal 128-col blocks (QK, exp, PV all sub-ranged). Softmax
uses exp without max subtraction (RMS-normed q,k bound |s| <= 8) in a
transposed scores layout [kpos, q]. Denominators come from a ones block
appended to V; normalization divides the small per-head attention output
using reciprocal_approx_fast. RMS mean-squares are computed from pre-rotary
values (rotation preserves norms), in parallel with the rotation itself.
"""
import numpy as np

N_EMBD = 1024
N_HEAD = 16
HD = 64
S = 1024
B = 4
N = B * S
NCORES = 8
HPC = 8             # heads per core
NHC = HPC // 2      # head-pair chunks per core
NB = S // 128       # row blocks per sequence
ND = N_EMBD // 128  # contraction chunks
JW = 3 * HPC * HD   # qkv feature width per core (1536)
NEG = -30000.0
RMS_EPS = 1.1920929e-07

_cached = {}
DEBUG = False


def _build():
    import concourse.bacc as bacc
    import concourse.mybir as mybir
    import concourse.tile as tile
    import concourse.bass as bass
    from concourse.masks import make_identity

    F32 = mybir.dt.float32
    BF16 = mybir.dt.bfloat16
    AF = mybir.ActivationFunctionType

    nc = bacc.Bacc('TRN2', target_bir_lowering=False, debug=False,
                   num_devices=NCORES)
    xt = nc.dram_tensor('xt', [N_EMBD, S], BF16, kind='ExternalInput').ap()
    wqkvT = nc.dram_tensor('wqkvT', [N_EMBD, JW], BF16, kind='ExternalInput').ap()
    woT = nc.dram_tensor('woT', [HPC * HD, N_EMBD], BF16, kind='ExternalInput').ap()
    cosg = nc.dram_tensor('cosg', [S, HD // 2], F32, kind='ExternalInput').ap()
    sing = nc.dram_tensor('sing', [S, HD // 2], F32, kind='ExternalInput').ap()
    ypart = nc.dram_tensor('ypart', [S // 2, N_EMBD], BF16, kind='ExternalOutput').ap()
    ystage = nc.dram_tensor('ystage', [S, N_EMBD], BF16).ap()
    yred = nc.dram_tensor('yred', [S // 2, N_EMBD], BF16).ap()
    if DEBUG:
        dbg_q = nc.dram_tensor('dbg_q', [128, NHC * S], BF16, kind='ExternalOutput').ap()
        dbg_k = nc.dram_tensor('dbg_k', [128, NHC * S], BF16, kind='ExternalOutput').ap()
        dbg_att = nc.dram_tensor('dbg_att', [128, NHC * S], BF16, kind='ExternalOutput').ap()
        dbg_v = nc.dram_tensor('dbg_v', [128, NB * HPC * 128], BF16, kind='ExternalOutput').ap()
        dbg_ys = nc.dram_tensor('dbg_ys', [S, N_EMBD], BF16, kind='ExternalOutput').ap()

    def bcast_mid(t, n, width):
        # view [128, width] tile as [128, n, width] broadcasting over middle dim
        return bass.AP(tensor=t.tensor, offset=t.offset,
                       ap=[t.ap[0], [0, n], t.ap[-1]])

    def bcast_last(t, width):
        # view [128, n] tile as [128, n, width] broadcasting over last dim
        return bass.AP(tensor=t.tensor, offset=t.offset,
                       ap=[t.ap[0], t.ap[1], [0, width]])

    def view3(t, off, n, w, stride=None):
        # view [128, C] contiguous tile as [128, n, w] starting at column off
        return bass.AP(tensor=t.tensor, offset=t.offset + off,
                       ap=[t.ap[0], [w if stride is None else stride, n], [1, w]])

    with tile.TileContext(nc) as tc:
        import contextlib
        ctx = contextlib.ExitStack()
        with ctx:
            const = ctx.enter_context(tc.tile_pool(name='const', bufs=1))
            persist = ctx.enter_context(tc.tile_pool(name='persist', bufs=1))

            identb = const.tile([128, 128], BF16)
            make_identity(nc, identb)
            epst = const.tile([128, 1], F32)
            nc.vector.memset(epst, RMS_EPS)

            # persistent attention operands, bf16
            qT = persist.tile([128, NHC, S], BF16, name='qT')
            kT = persist.tile([128, NHC, S], BF16, name='kT')
            attT = persist.tile([128, NHC, S], BF16, name='attT')
            # v per head padded to 128 cols: cols 0:64 = v, 64:128 = ones, so
            # the PV matmul also produces replicated denominator rows.
            vt = [persist.tile([128, HPC, 128], BF16, name=f'vt{i}') for i in range(NB)]

            cost = [const.tile([128, HD // 2], F32, name=f'cos{i}') for i in range(NB)]
            sint = [const.tile([128, HD // 2], F32, name=f'sin{i}') for i in range(NB)]
            for nb in range(NB):
                nc.sync.dma_start(out=cost[nb], in_=cosg[nb * 128:(nb + 1) * 128])
                nc.sync.dma_start(out=sint[nb], in_=sing[nb * 128:(nb + 1) * 128])
            for nb in range(NB):
                nc.gpsimd.memset(vt[nb][:, :, HD:128], 1.0)

            # ---- phase 1: QKV projection + rotary/RMS + head transposes ----
            with tc.tile_pool(name='xwp', bufs=1) as xwp, \
                 tc.tile_pool(name='work', bufs=2) as work, \
                 tc.tile_pool(name='psq', bufs=2, space='PSUM') as psq, \
                 tc.tile_pool(name='pst', bufs=2, space='PSUM') as pst:
                xT = []
                wq = []
                for d in range(ND):
                    xTd = xwp.tile([128, S], BF16, name=f'xT{d}')
                    nc.sync.dma_start(out=xTd, in_=xt[d * 128:(d + 1) * 128])
                    xT.append(xTd)
                    wqd = xwp.tile([128, JW], BF16, name=f'wq{d}')
                    nc.sync.dma_start(out=wqd, in_=wqkvT[d * 128:(d + 1) * 128])
                    wq.append(wqd)

                for nb in range(NB):
                    pq = psq.tile([128, 3, 512], F32, tag='pq')
                    for d in range(ND):
                        for jc in range(3):
                            nc.tensor.matmul(
                                pq[:, jc, :],
                                xT[d][:, nb * 128:(nb + 1) * 128],
                                wq[d][:, jc * 512:(jc + 1) * 512],
                                start=(d == 0), stop=(d == ND - 1))

                    # mean-square from pre-rotary qk (rotation preserves norms)
                    qkv = view3(pq, 0, 16, 64)     # [128, 16 heads, 64] q|k
                    x1 = view3(pq, 0, 16, 32, stride=64)
                    x2 = view3(pq, 32, 16, 32, stride=64)
                    sq = work.tile([128, 16, 64], F32, tag='sq')
                    nc.scalar.activation(out=sq, in_=qkv, func=AF.Square)
                    ms = work.tile([128, 16], F32, tag='ms')
                    nc.vector.reduce_sum(out=ms, in_=sq, axis=mybir.AxisListType.X)
                    nc.scalar.activation(out=ms, in_=ms, func=AF.Sqrt,
                                         bias=epst, scale=1.0 / HD)
                    nc.vector.reciprocal(out=ms, in_=ms)
                    # fold the attention scale into q's normalizer
                    nc.scalar.mul(out=ms[:, 0:HPC], in_=ms[:, 0:HPC], mul=HD ** -0.5)

                    cb = bcast_mid(cost[nb], 16, 32)
                    sb = bcast_mid(sint[nb], 16, 32)
                    ta = work.tile([128, 16, 32], F32, tag='ta')
                    tb = work.tile([128, 16, 32], F32, tag='tb')
                    rot = work.tile([128, 16, 64], F32, tag='rot')
                    nc.vector.tensor_mul(ta, x1, cb)
                    nc.vector.tensor_mul(tb, x2, sb)
                    nc.vector.tensor_add(rot[:, :, 0:32], ta, tb)
                    nc.vector.tensor_mul(ta, x2, cb)
                    nc.vector.tensor_mul(tb, x1, sb)
                    nc.vector.tensor_tensor(out=rot[:, :, 32:64], in0=ta, in1=tb,
                                            op=mybir.AluOpType.subtract)
                    qkb = work.tile([128, 16, 64], BF16, tag='qkb')
                    nc.vector.tensor_mul(qkb, rot, bcast_last(ms, 64))

                    # v: psum f32 -> bf16 sbuf
                    nc.scalar.copy(out=vt[nb][:, :, 0:HD], in_=view3(pq, 1024, 8, 64))

                    # transpose q,k head pairs: [pos, 2hd] -> [2hd, pos]
                    tp = pst.tile([128, 8, 128], BF16, tag='tp')
                    for g in range(8):
                        nc.tensor.transpose(
                            tp[:, g, :],
                            qkb[:, 2 * g:2 * g + 2, :].rearrange("p a b -> p (a b)"),
                            identb)
                    nc.vector.tensor_copy(qT[:, :, nb * 128:(nb + 1) * 128], tp[:, 0:4, :])
                    nc.scalar.copy(out=kT[:, :, nb * 128:(nb + 1) * 128], in_=tp[:, 4:8, :])

            # ---- phase 2: attention interleaved with output projection ----
            with tc.tile_pool(name='estp', bufs=4) as estp, \
                 tc.tile_pool(name='attw', bufs=2) as attw, \
                 tc.tile_pool(name='wop', bufs=1) as wop, \
                 tc.tile_pool(name='ywork', bufs=3) as yw, \
                 tc.tile_pool(name='pssc', bufs=2, space='PSUM') as pssc, \
                 tc.tile_pool(name='pspv', bufs=1, space='PSUM') as pspv, \
                 tc.tile_pool(name='psy', bufs=2, space='PSUM') as psy:
                wo = []
                for f in range(NHC):
                    wof = wop.tile([128, N_EMBD], BF16, name=f'wo{f}')
                    nc.sync.dma_start(out=wof, in_=woT[f * 128:(f + 1) * 128])
                    wo.append(wof)

                for qg in range(2):
                    for hc in range(NHC):
                        nkc = 4 + qg * 4
                        pvt = pspv.tile([128, 2, 512], F32, tag='pv')
                        for kc in range(nkc):
                            vs = max(0, kc - qg * 4) * 128
                            diag = kc >= qg * 4
                            sct = pssc.tile([128, 2, 512], F32, tag='sc')
                            for h2 in range(2):
                                nc.tensor.matmul(
                                    sct[:, h2, vs:],
                                    kT[h2 * HD:(h2 + 1) * HD, hc, kc * 128:(kc + 1) * 128],
                                    qT[h2 * HD:(h2 + 1) * HD, hc,
                                       qg * 512 + vs:(qg + 1) * 512],
                                    start=True, stop=True,
                                    tile_position=(h2 * HD, 0))
                            est = estp.tile([128, 2, 512], BF16, tag='est')
                            nc.scalar.activation(out=est[:, :, vs:], in_=sct[:, :, vs:],
                                                 func=AF.Exp)
                            if diag:
                                # zero the above-diagonal entries (k > q) of
                                # the diagonal 128-col block, per head
                                nc.gpsimd.affine_select(
                                    out=est[:, :, vs:vs + 128],
                                    in_=est[:, :, vs:vs + 128],
                                    compare_op=mybir.AluOpType.is_ge,
                                    fill=0.0, base=0, pattern=[[0, 2], [1, 128]],
                                    channel_multiplier=-1)
                            for h2 in range(2):
                                nc.tensor.matmul(
                                    pvt[:, h2, vs:], vt[kc][:, hc * 2 + h2],
                                    est[:, h2, vs:],
                                    start=(kc == 0), stop=(kc == nkc - 1),
                                    skip_group_check=True)
                        den = attw.tile([HD, 2, 512], F32, tag='den')
                        nc.vector.reciprocal(out=den, in_=pvt[HD:128, :, :])
                        for h2 in range(2):
                            nc.vector.tensor_mul(
                                attT[h2 * HD:(h2 + 1) * HD, hc, qg * 512:(qg + 1) * 512],
                                pvt[0:HD, h2, :], den[:, h2, :])

                    # project this half's rows while the other half's attention runs
                    for qt in range(qg * 4, qg * 4 + 4):
                        for og in range(2):
                            py = psy.tile([128, 512], F32, tag='py')
                            for f in range(NHC):
                                nc.tensor.matmul(
                                    py,
                                    attT[:, f, qt * 128:(qt + 1) * 128],
                                    wo[f][:, og * 512:(og + 1) * 512],
                                    start=(f == 0), stop=(f == NHC - 1))
                            ys = yw.tile([128, 512], BF16, tag='ys')
                            nc.vector.tensor_copy(ys, py)
                            nc.sync.dma_start(
                                out=ystage[qt * 128:(qt + 1) * 128,
                                           og * 512:(og + 1) * 512],
                                in_=ys)
                    if DEBUG and qg == 1:
                        nc.sync.dma_start(out=dbg_q, in_=qT.rearrange("p a b -> p (a b)"))
                        nc.sync.dma_start(out=dbg_k, in_=kT.rearrange("p a b -> p (a b)"))
                        nc.sync.dma_start(out=dbg_att, in_=attT.rearrange("p a b -> p (a b)"))
                        for nb in range(NB):
                            nc.sync.dma_start(
                                out=dbg_v[:, nb * 1024:(nb + 1) * 1024],
                                in_=vt[nb].rearrange("p a b -> p (a b)"))
                        nc.sync.dma_start(out=dbg_ys, in_=ystage)
                    rs = slice(qg * 512, (qg + 1) * 512)
                    nc.gpsimd.collective_compute(
                        "ReduceScatter", mybir.AluOpType.add,
                        replica_groups=[[0, 1], [2, 3], [4, 5], [6, 7]],
                        ins=[ystage[rs, :]],
                        outs=[yred[qg * 256:(qg + 1) * 256, :]])
                    nc.sync.dma_start(out=ypart[qg * 256:(qg + 1) * 256, :],
                                      in_=yred[qg * 256:(qg + 1) * 256, :])

    nc.compile()
    return nc


def _get_nc():
    if 'nc' not in _cached:
        _cached['nc'] = _build()
    return _cached['nc']


def kernel(x, Wqkv, Wo, cos_cache, sin_cache, cu_seqlens, position_ids,
           max_seqlen, **_ignored):
    import ml_dtypes
    from concourse.bass_utils import run_bass_kernel_spmd

    BF = ml_dtypes.bfloat16
    x = np.asarray(x)
    Wqkv = np.asarray(Wqkv)
    Wo = np.asarray(Wo)
    cos_cache = np.asarray(cos_cache, dtype=np.float32)
    sin_cache = np.asarray(sin_cache, dtype=np.float32)
    position_ids = np.asarray(position_ids)

    nc = _get_nc()
    in_maps = []
    for c in range(NCORES):
        b, hh = c // 2, c % 2
        rows = slice(b * S, (b + 1) * S)
        qsl = slice(hh * HPC * HD, (hh + 1) * HPC * HD)
        ksl = slice(N_EMBD + hh * HPC * HD, N_EMBD + (hh + 1) * HPC * HD)
        vsl = slice(2 * N_EMBD + hh * HPC * HD, 2 * N_EMBD + (hh + 1) * HPC * HD)
        wqkvT_c = np.concatenate(
            [Wqkv[qsl], Wqkv[ksl], Wqkv[vsl]], axis=0).T
        woT_c = Wo[:, qsl].T
        pos = position_ids[rows]
        in_maps.append({
            'xt': np.ascontiguousarray(x[rows].T.astype(BF)),
            'wqkvT': np.ascontiguousarray(wqkvT_c.astype(BF)),
            'woT': np.ascontiguousarray(woT_c.astype(BF)),
            'cosg': np.ascontiguousarray(cos_cache[pos]),
            'sing': np.ascontiguousarray(sin_cache[pos]),
        })

    r = run_bass_kernel_spmd(nc, in_maps, list(range(NCORES)))
    out = np.empty((N, N_EMBD), dtype=np.float32)
    for b in range(B):
        for qg in range(2):
            for half in range(2):
                dst = b * S + qg * 512 + half * 256
                src = r.results[2 * b + half]['ypart'][qg * 256:(qg + 1) * 256]
                out[dst:dst + 256] = src.astype(np.float32)
    _cached['last_results'] = r
    return out


# revision 20
# speedup vs baseline: 1.9702x; 1.2853x over previous
"""Causal varlen self-attention (packed equal-length sequences) on 8 trn2 cores.

Sharding: 4 sequences x 2 head-groups. Core c handles sequence b = c//2 and
heads hh*8..hh*8+8 where hh = c%2. Each core computes the QKV projection for
its sequence restricted to its heads, rotary+RMSNorm, causal attention over
its 8 heads, and a partial output projection over its 512 features. The pair
of cores for a sequence ReduceScatter their partial y (each ends with half
the reduced rows); the host stitches the halves.

All matmuls run in bf16 (rel err ~4e-3, tolerance 2e-2). The host ships x
pre-transposed and weights pre-converted to bf16, so the device does no
f32 casts for weights and no x transpose. Attention computes only the
at-or-below-diagonal 128-col blocks (QK, exp, PV all sub-ranged). Softmax
uses exp without max subtraction (RMS-normed q,k bound |s| <= 8) in a
transposed scores layout [kpos, q]. Denominators come from a ones block
appended to V; normalization divides the small per-head attention output
using reciprocal_approx_fast. RMS mean-squares are computed from pre-rotary
values (rotation preserves norms), in parallel with the rotation itself.
"""
import numpy as np

N_EMBD = 1024
N_HEAD = 16
HD = 64
S = 1024
B = 4
N = B * S
NCORES = 8
HPC = 8             # heads per core
NHC = HPC // 2      # head-pair chunks per core
NB = S // 128       # row blocks per sequence
ND = N_EMBD // 128  # contraction chunks
JW = 3 * HPC * HD   # qkv feature width per core (1536)
NEG = -30000.0
RMS_EPS = 1.1920929e-07

_cached = {}
DEBUG = False


def _build():
    import concourse.bacc as bacc
    import concourse.mybir as mybir
    import concourse.tile as tile
    import concourse.bass as bass
    from concourse.masks import make_identity

    F32 = mybir.dt.float32
    BF16 = mybir.dt.bfloat16
    AF = mybir.ActivationFunctionType

    nc = bacc.Bacc('TRN2', target_bir_lowering=False, debug=False,
                   num_devices=NCORES)
    xt = nc.dram_tensor('xt', [N_EMBD, S], BF16, kind='ExternalInput').ap()
    wqkvT = nc.dram_tensor('wqkvT', [N_EMBD, JW], BF16, kind='ExternalInput').ap()
    woT = nc.dram_tensor('woT', [HPC * HD, N_EMBD], BF16, kind='ExternalInput').ap()
    cosg = nc.dram_tensor('cosg', [S, HD // 2], BF16, kind='ExternalInput').ap()
    sing = nc.dram_tensor('sing', [S, HD // 2], BF16, kind='ExternalInput').ap()
    ypart = nc.dram_tensor('ypart', [S // 2, N_EMBD], BF16, kind='ExternalOutput').ap()
    ystage = nc.dram_tensor('ystage', [S, N_EMBD], BF16).ap()
    yred = nc.dram_tensor('yred', [S // 2, N_EMBD], BF16).ap()
    if DEBUG:
        dbg_q = nc.dram_tensor('dbg_q', [128, NHC * S], BF16, kind='ExternalOutput').ap()
        dbg_k = nc.dram_tensor('dbg_k', [128, NHC * S], BF16, kind='ExternalOutput').ap()
        dbg_att = nc.dram_tensor('dbg_att', [128, NHC * S], BF16, kind='ExternalOutput').ap()
        dbg_v = nc.dram_tensor('dbg_v', [128, NB * HPC * 128], BF16, kind='ExternalOutput').ap()
        dbg_ys = nc.dram_tensor('dbg_ys', [S, N_EMBD], BF16, kind='ExternalOutput').ap()

    def bcast_mid(t, n, width):
        # view [128, width] tile as [128, n, width] broadcasting over middle dim
        return bass.AP(tensor=t.tensor, offset=t.offset,
                       ap=[t.ap[0], [0, n], t.ap[-1]])

    def bcast_last(t, width):
        # view [128, n] tile as [128, n, width] broadcasting over last dim
        return bass.AP(tensor=t.tensor, offset=t.offset,
                       ap=[t.ap[0], t.ap[1], [0, width]])

    def view3(t, off, n, w, stride=None):
        # view [128, C] contiguous tile as [128, n, w] starting at column off
        return bass.AP(tensor=t.tensor, offset=t.offset + off,
                       ap=[t.ap[0], [w if stride is None else stride, n], [1, w]])

    with tile.TileContext(nc) as tc:
        import contextlib
        ctx = contextlib.ExitStack()
        with ctx:
            const = ctx.enter_context(tc.tile_pool(name='const', bufs=1))
            persist = ctx.enter_context(tc.tile_pool(name='persist', bufs=1))

            identb = const.tile([128, 128], BF16)
            make_identity(nc, identb)
            # additive causal maskT[k, q] = 0 if k <= q else NEG
            maskT = const.tile([128, 128], F32)
            nc.gpsimd.memset(maskT, 0.0)
            nc.gpsimd.affine_select(
                out=maskT, in_=maskT, compare_op=mybir.AluOpType.is_ge,
                fill=NEG, base=0, pattern=[[1, 128]], channel_multiplier=-1)
            epst = const.tile([128, 1], F32)
            nc.vector.memset(epst, RMS_EPS)

            # persistent attention operands, bf16
            qT = persist.tile([128, NHC, S], BF16, name='qT')
            kT = persist.tile([128, NHC, S], BF16, name='kT')
            attT = persist.tile([128, NHC, S], BF16, name='attT')
            # v per head padded to 128 cols: cols 0:64 = ones, 64:128 = v, so
            # the PV matmul produces denominator rows at partitions 0:64
            # (reciprocal_approx_fast needs an offset-0 view).
            vt = [persist.tile([128, HPC, 128], BF16, name=f'vt{i}') for i in range(NB)]

            cost = [const.tile([128, HD // 2], BF16, name=f'cos{i}') for i in range(NB)]
            sint = [const.tile([128, HD // 2], BF16, name=f'sin{i}') for i in range(NB)]
            for nb in range(NB):
                nc.sync.dma_start(out=cost[nb], in_=cosg[nb * 128:(nb + 1) * 128])
                nc.sync.dma_start(out=sint[nb], in_=sing[nb * 128:(nb + 1) * 128])
            for nb in range(NB):
                nc.gpsimd.memset(vt[nb][:, :, 0:HD], 1.0)

            # ---- phase 1: QKV projection + rotary/RMS + head transposes ----
            with tc.tile_pool(name='xwp', bufs=1) as xwp, \
                 tc.tile_pool(name='work', bufs=2) as work, \
                 tc.tile_pool(name='psq', bufs=2, space='PSUM') as psq, \
                 tc.tile_pool(name='pst', bufs=2, space='PSUM') as pst:
                xT = []
                wq = []
                for d in range(ND):
                    xTd = xwp.tile([128, S], BF16, name=f'xT{d}')
                    nc.sync.dma_start(out=xTd, in_=xt[d * 128:(d + 1) * 128])
                    xT.append(xTd)
                    wqd = xwp.tile([128, JW], BF16, name=f'wq{d}')
                    nc.sync.dma_start(out=wqd, in_=wqkvT[d * 128:(d + 1) * 128])
                    wq.append(wqd)

                for nb in range(NB):
                    pq = psq.tile([128, 3, 512], F32, tag='pq')
                    for d in range(ND):
                        for jc in range(3):
                            nc.tensor.matmul(
                                pq[:, jc, :],
                                xT[d][:, nb * 128:(nb + 1) * 128],
                                wq[d][:, jc * 512:(jc + 1) * 512],
                                start=(d == 0), stop=(d == ND - 1))

                    # qk to bf16 sbuf; mean-square from pre-rotary values
                    # (rotation preserves norms)
                    qksb = work.tile([128, 16, 64], BF16, tag='qksb')
                    nc.scalar.copy(out=qksb, in_=view3(pq, 0, 16, 64))
                    sq = work.tile([128, 16, 64], BF16, tag='sq')
                    nc.scalar.activation(out=sq, in_=qksb, func=AF.Square)
                    ms = work.tile([128, 16], F32, tag='ms')
                    nc.vector.reduce_sum(out=ms, in_=sq, axis=mybir.AxisListType.X)
                    nc.scalar.activation(out=ms, in_=ms, func=AF.Sqrt,
                                         bias=epst, scale=1.0 / HD)
                    msb = work.tile([128, 16], BF16, tag='msb')
                    with nc.allow_low_precision(reason="bf16 rms normalizer"):
                        nc.vector.reciprocal(out=msb, in_=ms)

                    x1 = qksb[:, :, 0:32]
                    x2 = qksb[:, :, 32:64]
                    cb = bcast_mid(cost[nb], 16, 32)
                    sb = bcast_mid(sint[nb], 16, 32)
                    ta = work.tile([128, 16, 32], BF16, tag='ta')
                    tb = work.tile([128, 16, 32], BF16, tag='tb')
                    rot = work.tile([128, 16, 64], BF16, tag='rot')
                    nc.vector.tensor_mul(ta, x1, cb)
                    nc.vector.tensor_mul(tb, x2, sb)
                    nc.vector.tensor_add(rot[:, :, 0:32], ta, tb)
                    nc.vector.tensor_mul(ta, x2, cb)
                    nc.vector.tensor_mul(tb, x1, sb)
                    nc.vector.tensor_tensor(out=rot[:, :, 32:64], in0=ta, in1=tb,
                                            op=mybir.AluOpType.subtract)
                    qkb = work.tile([128, 16, 64], BF16, tag='qkb')
                    nc.vector.tensor_mul(qkb, rot, bcast_last(msb, 64))

                    # v: psum f32 -> bf16 sbuf (second half of the padded tile)
                    nc.scalar.copy(out=vt[nb][:, :, HD:128], in_=view3(pq, 1024, 8, 64))

                    # transpose q,k head pairs: [pos, 2hd] -> [2hd, pos]
                    tp = pst.tile([128, 8, 128], BF16, tag='tp')
                    for g in range(8):
                        nc.tensor.transpose(
                            tp[:, g, :],
                            qkb[:, 2 * g:2 * g + 2, :].rearrange("p a b -> p (a b)"),
                            identb)
                    nc.vector.tensor_copy(qT[:, :, nb * 128:(nb + 1) * 128], tp[:, 0:4, :])
                    nc.scalar.copy(out=kT[:, :, nb * 128:(nb + 1) * 128], in_=tp[:, 4:8, :])

            # ---- phase 2: attention interleaved with output projection ----
            with tc.tile_pool(name='estp', bufs=4) as estp, \
                 tc.tile_pool(name='attw', bufs=2) as attw, \
                 tc.tile_pool(name='wop', bufs=1) as wop, \
                 tc.tile_pool(name='ywork', bufs=3) as yw, \
                 tc.tile_pool(name='pssc', bufs=2, space='PSUM') as pssc, \
                 tc.tile_pool(name='pspv', bufs=1, space='PSUM') as pspv, \
                 tc.tile_pool(name='psy', bufs=2, space='PSUM') as psy:
                wo = []
                for f in range(NHC):
                    wof = wop.tile([128, N_EMBD], BF16, name=f'wo{f}')
                    nc.sync.dma_start(out=wof, in_=woT[f * 128:(f + 1) * 128])
                    wo.append(wof)

                for qg in range(2):
                    for hc in range(NHC):
                        nkc = 4 + qg * 4
                        pvt = pspv.tile([128, 2, 512], F32, tag='pv')
                        for kc in range(nkc):
                            vs = max(0, kc - qg * 4) * 128
                            diag = kc >= qg * 4
                            sct = pssc.tile([128, 2, 512], F32, tag='sc')
                            for h2 in range(2):
                                nc.tensor.matmul(
                                    sct[:, h2, vs:],
                                    kT[h2 * HD:(h2 + 1) * HD, hc, kc * 128:(kc + 1) * 128],
                                    qT[h2 * HD:(h2 + 1) * HD, hc,
                                       qg * 512 + vs:(qg + 1) * 512],
                                    start=True, stop=True,
                                    tile_position=(h2 * HD, 0))
                            if diag:
                                nc.vector.tensor_add(
                                    sct[:, :, vs:vs + 128], sct[:, :, vs:vs + 128],
                                    bcast_mid(maskT, 2, 128))
                            est = estp.tile([128, 2, 512], BF16, tag='est')
                            # attention scale D^-0.5 folded into the exp
                            nc.scalar.activation(out=est[:, :, vs:], in_=sct[:, :, vs:],
                                                 func=AF.Exp, scale=HD ** -0.5)
                            for h2 in range(2):
                                nc.tensor.matmul(
                                    pvt[:, h2, vs:], vt[kc][:, hc * 2 + h2],
                                    est[:, h2, vs:],
                                    start=(kc == 0), stop=(kc == nkc - 1),
                                    skip_group_check=True)
                        den = attw.tile([HD, 2, 512], F32, tag='den')
                        nc.vector.reciprocal_approx_fast(den, pvt[0:HD, :, :])
                        for h2 in range(2):
                            nc.vector.tensor_mul(
                                attT[h2 * HD:(h2 + 1) * HD, hc, qg * 512:(qg + 1) * 512],
                                pvt[HD:128, h2, :], den[:, h2, :])

                    # project this half's rows while the other half's attention runs
                    for qt in range(qg * 4, qg * 4 + 4):
                        for og in range(2):
                            py = psy.tile([128, 512], F32, tag='py')
                            for f in range(NHC):
                                nc.tensor.matmul(
                                    py,
                                    attT[:, f, qt * 128:(qt + 1) * 128],
                                    wo[f][:, og * 512:(og + 1) * 512],
                                    start=(f == 0), stop=(f == NHC - 1))
                            ys = yw.tile([128, 512], BF16, tag='ys')
                            if og == 0:
                                nc.vector.tensor_copy(ys, py)
                            else:
                                nc.scalar.copy(out=ys, in_=py)
                            nc.sync.dma_start(
                                out=ystage[qt * 128:(qt + 1) * 128,
                                           og * 512:(og + 1) * 512],
                                in_=ys)
                    if DEBUG and qg == 1:
                        nc.sync.dma_start(out=dbg_q, in_=qT.rearrange("p a b -> p (a b)"))
                        nc.sync.dma_start(out=dbg_k, in_=kT.rearrange("p a b -> p (a b)"))
                        nc.sync.dma_start(out=dbg_att, in_=attT.rearrange("p a b -> p (a b)"))
                        for nb in range(NB):
                            nc.sync.dma_start(
                                out=dbg_v[:, nb * 1024:(nb + 1) * 1024],
                                in_=vt[nb].rearrange("p a b -> p (a b)"))
                        nc.sync.dma_start(out=dbg_ys, in_=ystage)
                    rs = slice(qg * 512, (qg + 1) * 512)
                    nc.gpsimd.collective_compute(
                        "ReduceScatter", mybir.AluOpType.add,
                        replica_groups=[[0, 1], [2, 3], [4, 5], [6, 7]],
                        ins=[ystage[rs, :]],
                        outs=[yred[qg * 256:(qg + 1) * 256, :]])
                    nc.sync.dma_start(out=ypart[qg * 256:(qg + 1) * 256, :],
                                      in_=yred[qg * 256:(qg + 1) * 256, :])

    nc.compile()
    return nc


def _get_nc():
    if 'nc' not in _cached:
        _cached['nc'] = _build()
    return _cached['nc']


def kernel(x, Wqkv, Wo, cos_cache, sin_cache, cu_seqlens, position_ids,
           max_seqlen, **_ignored):
    import ml_dtypes
    from concourse.bass_utils import run_bass_kernel_spmd

    BF = ml_dtypes.bfloat16
    x = np.asarray(x)
    Wqkv = np.asarray(Wqkv)
    Wo = np.asarray(Wo)
    cos_cache = np.asarray(cos_cache, dtype=np.float32)
    sin_cache = np.asarray(sin_cache, dtype=np.float32)
    position_ids = np.asarray(position_ids)

    nc = _get_nc()
    in_maps = []
    for c in range(NCORES):
        b, hh = c // 2, c % 2
        rows = slice(b * S, (b + 1) * S)
        qsl = slice(hh * HPC * HD, (hh + 1) * HPC * HD)
        ksl = slice(N_EMBD + hh * HPC * HD, N_EMBD + (hh + 1) * HPC * HD)
        vsl = slice(2 * N_EMBD + hh * HPC * HD, 2 * N_EMBD + (hh + 1) * HPC * HD)
        wqkvT_c = np.concatenate(
            [Wqkv[qsl], Wqkv[ksl], Wqkv[vsl]], axis=0).T
        woT_c = Wo[:, qsl].T
        pos = position_ids[rows]
        in_maps.append({
            'xt': np.ascontiguousarray(x[rows].T.astype(BF)),
            'wqkvT': np.ascontiguousarray(wqkvT_c.astype(BF)),
            'woT': np.ascontiguousarray(woT_c.astype(BF)),
            'cosg': np.ascontiguousarray(cos_cache[pos].astype(BF)),
            'sing': np.ascontiguousarray(sin_cache[pos].astype(BF)),
        })

    r = run_bass_kernel_spmd(nc, in_maps, list(range(NCORES)))
    out = np.empty((N, N_EMBD), dtype=np.float32)
    for b in range(B):
        for qg in range(2):
            for half in range(2):
                dst = b * S + qg * 512 + half * 256
                src = r.results[2 * b + half]['ypart'][qg * 256:(qg + 1) * 256]
                out[dst:dst + 256] = src.astype(np.float32)
    _cached['last_results'] = r
    return out
